# revision 41
# baseline (speedup 1.0000x reference)
"""GAT-style masked attention kernel for Trainium2 (8 NeuronCores, SPMD).

Problem: out = softmax(mask(x @ x^T / sqrt(D), adj)) @ x
  x:   [B=32, N=2048, D=64] f32
  adj: [N, N] int32 (0/1 mask, broadcast over batch)

Strategy: data-parallel over B (4 batches/core). Per batch, compute the
score matrix TRANSPOSED (E^T[k, q] layout; valid because x@x^T is
symmetric), so the second matmul (attn @ V) needs no on-chip transpose of
the big matrix. The softmax max-subtraction is replaced by a per-query
upper-bound shift folded into the QK^T contraction (see _prep_in_maps),
and the adjacency mask is applied multiplicatively post-exp (fp16, DVE 2x
mode). Row sums come for free by augmenting V with a ones column; the
normalization happens on the small [65, q] output after a PE transpose.

QK^T runs in fp32r (TF32-like, full PE rate at N>=256, ~1.6e-4 rel err);
exp values and AV run in fp16 (~5e-4 rel err, errors in the softmax ratio
largely cancel since numerator and denominator share the same E values).
"""

import numpy as np

import concourse.bacc as bacc
import concourse.tile as tile
import concourse.masks as masks
from concourse import mybir
from concourse.bass_utils import run_bass_kernel_spmd

B, N, D = 32, 2048, 64
NCORES = 8
BPC = B // NCORES          # batches per core
QB = 1024                  # q block width (2 PSUM banks)
KT = 128                   # k tile height (partition dim)
NQB = N // QB
NKT = N // KT
SCALE = 1.0 / np.sqrt(D)   # 0.125

f32 = mybir.dt.float32
f32r = mybir.dt.float32r
fp16 = mybir.dt.float16

_cache = {}


def _build():
    if "nc" in _cache:
        return _cache["nc"]
    nc = bacc.Bacc("TRN2", target_bir_lowering=False, debug=False,
                   num_devices=NCORES)
    # xtl = [x_b^T; 1], xtr = [x_b^T; -bias_q]: contracting over 65 rows
    # computes x_k . x_q - bias_q, folding the per-query softmax shift
    # (needed to keep exp() in fp16 range) into the QK^T matmul for free.
    # Declared f32r so the fp32r matmul can consume the DMA'd bits directly.
    xtl = nc.dram_tensor("xtl", [BPC, D + 1, N], f32r, kind="ExternalInput")
    xtr = nc.dram_tensor("xtr", [BPC, D + 1, N], f32r, kind="ExternalInput")
    # va is pre-arranged on the host as [p, chunk, m] so each SBUF partition
    # reads one contiguous 4KB run (va[b, p, c, m] = x_aug[b, c*128+p, m]).
    va = nc.dram_tensor("va", [BPC, KT, N // KT, D + 1], fp16,
                        kind="ExternalInput")
    mt = nc.dram_tensor("mt", [N, N], fp16, kind="ExternalInput")
    out = nc.dram_tensor("out", [BPC, N, D], f32, kind="ExternalOutput")
    xtl_ap, xtr_ap, va_ap, mt_ap, out_ap = (
        xtl.ap(), xtr.ap(), va.ap(), mt.ap(), out.ap())

    with tile.TileContext(nc) as tc:
        with (
            tc.tile_pool(name="singles", bufs=1) as singles,
            tc.tile_pool(name="xr", bufs=2) as xr_pool,
            tc.tile_pool(name="vr", bufs=2) as vr_pool,
            tc.tile_pool(name="ef", bufs=3) as ef_pool,
            tc.tile_pool(name="er", bufs=3) as er_pool,
            tc.tile_pool(name="ob", bufs=2) as ob_pool,
            tc.tile_pool(name="ot", bufs=3) as ot_pool,
            tc.tile_pool(name="ps_s", bufs=2, space="PSUM") as ps_s_pool,
            tc.tile_pool(name="ps_o", bufs=1, space="PSUM") as ps_o_pool,
            tc.tile_pool(name="ps_t", bufs=2, space="PSUM") as ps_t_pool,
        ):
            def load_batch(b):
                # halves as separate tiles -> the first matmuls only wait
                # on the first 0.26MB chunk, not the full 1MB of x data
                xtl_r, xtr_r = [], []
                for h in range(2):
                    tl = xr_pool.tile([D + 1, N // 2], f32r, tag=f"xtl_r{h}")
                    nc.sync.dma_start(
                        out=tl, in_=xtl_ap[b][:, h * (N // 2):(h + 1) * (N // 2)])
                    xtl_r.append(tl)
                    tr = xr_pool.tile([D + 1, N // 2], f32r, tag=f"xtr_r{h}")
                    nc.sync.dma_start(
                        out=tr, in_=xtr_ap[b][:, h * (N // 2):(h + 1) * (N // 2)])
                    xtr_r.append(tr)
                va_h = vr_pool.tile([KT, NKT, D + 1], fp16)
                nc.sync.dma_start(out=va_h, in_=va_ap[b])
                return xtl_r, xtr_r, va_h

            # Batch 0 inputs go on the DMA queues ahead of the 8MiB mask so
            # the compute pipeline starts immediately; the mask is loaded in
            # [128, QB] chunks ordered q-block-0-first to stay ahead of the
            # mask-multiply consumers.
            b0_tiles = load_batch(0)
            mt_r = mt_ap.rearrange("(t p) q -> t p q", p=KT)
            mt_sb = {}
            for h in range(NQB):
                for t in range(NKT):
                    mtile = singles.tile([KT, QB], fp16, tag=f"mt{t}_{h}")
                    nc.sync.dma_start(out=mtile,
                                      in_=mt_r[t][:, h * QB:(h + 1) * QB])
                    mt_sb[(t, h)] = mtile
            ident = singles.tile([128, 128], f32)
            masks.make_identity(nc, ident[:, :])

            def epilogue_tail(b, qb, ob, final=False):
                # normalize + transpose back to [q, D]; runs after the NEXT
                # block's matmuls in PE program order so PE never stalls.
                # All 8 q-tiles land in one SBUF tile -> single out DMA
                # (the per-qt DMA triggers serialize at ~625ns each).
                # The final epilogue IS the kernel tail: run the multiplies
                # on the then-idle ScalarE and ship the output in 2 halves.
                ot = ot_pool.tile([KT, QB // KT, D], f32)
                nq = QB // KT
                for qt in range(nq):
                    pt = ps_t_pool.tile([KT, D + 1], f32)
                    nc.tensor.transpose(
                        pt[:, :],
                        ob[:, qt * KT:(qt + 1) * KT],
                        ident[:D + 1, :D + 1])
                    rs = ot_pool.tile([KT, 1], f32, tag="rs")
                    nc.vector.reciprocal(rs, pt[:, D:D + 1])
                    if final:
                        nc.scalar.mul(ot[:, qt, :], pt[:, :D], rs)
                    else:
                        nc.vector.tensor_scalar_mul(ot[:, qt, :], pt[:, :D], rs)
                out_r = out_ap[b, qb * QB:(qb + 1) * QB, :].rearrange(
                    "(c p) d -> p c d", p=KT)
                if final:
                    nc.sync.dma_start(out=out_r[:, :nq // 2, :],
                                      in_=ot[:, :nq // 2, :])
                    nc.sync.dma_start(out=out_r[:, nq // 2:, :],
                                      in_=ot[:, nq // 2:, :])
                else:
                    nc.sync.dma_start(out=out_r, in_=ot[:, :, :])

            pending = None
            next_tiles = b0_tiles
            for b in range(BPC):
                xtl_r, xtr_r, va_h = next_tiles
                next_tiles = None

                for qb in range(NQB):
                    qs = slice(qb * QB, (qb + 1) * QB)
                    po = ps_o_pool.tile([D + 1, QB], f32)
                    for kt in range(NKT):
                        if kt == 8 and qb == NQB - 1 and b + 1 < BPC:
                            next_tiles = load_batch(b + 1)
                        ps = ps_s_pool.tile([KT, QB], f32)
                        for h in range(QB // 512):
                            q0 = qb * QB + h * 512
                            nc.tensor.matmul(
                                ps[:, h * 512:(h + 1) * 512],
                                xtl_r[kt // 8][:, (kt % 8) * KT:
                                               (kt % 8 + 1) * KT],
                                xtr_r[q0 // (N // 2)][:, q0 % (N // 2):
                                                      q0 % (N // 2) + 512],
                                start=True, stop=True)
                        ef = ef_pool.tile([KT, QB], fp16)
                        nc.scalar.activation(
                            ef[:, :], ps[:, :],
                            mybir.ActivationFunctionType.Exp, scale=float(SCALE))
                        er = er_pool.tile([KT, QB], fp16)
                        nc.vector.tensor_mul(er[:, :], ef[:, :],
                                             mt_sb[(kt, qb)][:, :])
                        for h in range(QB // 512):
                            nc.tensor.matmul(
                                po[:, h * 512:(h + 1) * 512],
                                va_h[:, kt, :],
                                er[:, h * 512:(h + 1) * 512],
                                start=(kt == 0), stop=(kt == NKT - 1))

                    ob = ob_pool.tile([D + 1, QB], f32)
                    nc.vector.tensor_copy(ob[:, :], po[:, :])
                    if pending is not None:
                        epilogue_tail(*pending)
                    pending = (b, qb, ob)
            epilogue_tail(*pending, final=True)
    nc.compile()
    _cache["nc"] = nc
    return nc


def _prep_in_maps(x, adj):
    x = np.ascontiguousarray(np.asarray(x, dtype=np.float32))
    adj = np.asarray(adj)
    mt = (adj.T > 0).astype(np.float16)
    ones = np.ones((BPC, N, 1), dtype=np.float16)
    # Per-query shift bias_q = ||x_q|| * max_k ||x_k|| - 64 (per batch).
    # s_qk - bias_q <= 64 by Cauchy-Schwarz, so exp((s-b)/8) <= e^8 stays
    # in fp16 range; softmax output is invariant to any per-q shift.
    nrm = np.linalg.norm(x, axis=2)                        # [B, N]
    bias = nrm * nrm.max(axis=1, keepdims=True) - 64.0     # [B, N]
    xT = x.transpose(0, 2, 1)                              # [B, D, N]
    xtl_full = np.concatenate(
        [xT, np.ones((B, 1, N), np.float32)], axis=1)      # [B, D+1, N]
    xtr_full = np.concatenate(
        [xT, -bias[:, None, :].astype(np.float32)], axis=1)
    x16 = x.astype(np.float16)
    in_maps = []
    for c in range(NCORES):
        s = slice(c * BPC, (c + 1) * BPC)
        va = np.concatenate([x16[s], ones], axis=2)     # [BPC, N, D+1]
        va = np.ascontiguousarray(
            va.reshape(BPC, N // 128, 128, D + 1).transpose(0, 2, 1, 3))
        in_maps.append({
            "xtl": np.ascontiguousarray(xtl_full[s]),
            "xtr": np.ascontiguousarray(xtr_full[s]),
            "va": va,
            "mt": mt,
        })
    return in_maps


def _run(x, adj, trace=False, **kwargs):
    nc = _build()
    in_maps = _prep_in_maps(x, adj)
    res = run_bass_kernel_spmd(nc, in_maps, core_ids=list(range(NCORES)),
                               trace=trace, **kwargs)
    out = np.concatenate([r["out"] for r in res.results], axis=0)
    return out, res


def kernel(input, adj):
    out, _ = _run(input, adj)
    return out
